# revision 11
# baseline (speedup 1.0000x reference)
import sys

import numpy as np

if "/opt/trn_rl_repo" not in sys.path:
    sys.path.insert(0, "/opt/trn_rl_repo")

import concourse.bass as bass
import concourse.tile as tile
from concourse import bacc, masks, mybir
from concourse.bass_utils import run_bass_kernel_spmd

B, T, E, H = 8, 512, 64, 128
FP = mybir.dt.float32
I32 = mybir.dt.int32
ALU = mybir.AluOpType
ACTF = mybir.ActivationFunctionType

NCOL = 1099
C_XT = 0       # (128, 256) x[b].T folded: rows 0:64 toks 0:256, rows 64:128 toks 256:512
C_WKV = 256    # (128, 128) [WK.T | WV.T], duplicated in both row halves
C_W2T = 384    # (128, 64)  W2[b].T
C_B1 = 448     # (128, 1)   b1[b]
C_BC = 449     # (128, 4)   Bc'[128j+p] at [p, j]
C_DC = 453     # (128, 4)   Dc'
C_W1T = 457    # rows 0:64  (64,128) W1[b].T
C_MW2 = 585    # rows 0:64  (64,129) [W2[b] | b2[b]]
C_SW2A = 714   # rows 0:64  (64,129) [SW2[b] | Sb2[b]]
C_W2N = 457    # rows 64:128 (64,128) W2[b]
C_B2 = 585     # rows 64:128 (64,1)  b2[b]
C_SW2B = 586   # rows 64:128 (64,129) [SW2[b] | Sb2[b]]
C_W1M = 843    # rows 0:65  (65,128) [W1[b].T ; b1[b]]
C_W1S = 971    # rows 0:65  (65,128) [SW1[b].T ; Sb1[b]]

MAGIC = float(0x5F3759DF)


def _coeffs():
    beta = np.float32(1.0 - 0.999)
    eta = np.float32(0.95)
    theta = np.float32(0.05)
    beta_vec = np.full(T, beta, np.float32)
    eta_vec = np.full(T, eta, np.float32)
    p_prefix = np.cumprod(beta_vec, dtype=np.float32)
    p_suffix = np.cumprod(beta_vec[::-1], dtype=np.float32)[::-1]
    q_prefix = np.cumprod(eta_vec, dtype=np.float32)
    q_suffix = np.cumprod(eta_vec[::-1], dtype=np.float32)[::-1]
    p_T = np.float32(p_prefix[-1])
    q_T = np.float32(q_prefix[-1])
    w = (p_suffix / beta_vec * q_prefix).astype(np.float32)
    A_T = np.float32(w.sum(dtype=np.float32))
    partial = np.cumsum(w[::-1], dtype=np.float32)[::-1].astype(np.float32)
    Bco = (-theta * partial / q_prefix).astype(np.float32)
    Dco = (-theta * q_suffix).astype(np.float32)
    s = np.float32(2.0 / 512.0)  # grad-of-mean factor, exact power of two
    return (Bco * s).astype(np.float32), (Dco * s).astype(np.float32), \
        float(p_T), float(A_T), float(q_T)


def _build_nc(stage=7):
    _, _, p_T, A_T, q_T = _coeffs()
    nc = bacc.Bacc(None, target_bir_lowering=False, debug=True)

    blob_d = nc.declare_dram_parameter("blob", [128, NCOL], FP, isOutput=False)
    w2pack_d = nc.declare_dram_parameter("w2pack", [128, 129], FP, isOutput=True)
    w1pack_d = nc.declare_dram_parameter("w1pack", [65, 256], FP, isOutput=True)
    loss_d = nc.declare_dram_parameter("loss", [128, 4], FP, isOutput=True)

    with tile.TileContext(nc) as tc:
        with (
            tc.tile_pool(name="sb", bufs=1) as sb,
            tc.tile_pool(name="ps", bufs=1, space=bass.MemorySpace.PSUM) as ps,
        ):
            blob = sb.tile([128, NCOL], FP)
            TK = sb.tile([128, T], FP)
            vT = sb.tile([128, T], FP)
            HT = sb.tile([128, T], FP)
            dsl = sb.tile([128, T], FP)
            dzT = sb.tile([128, T], FP)
            sqs = sb.tile([128, T], FP)
            ssq = sb.tile([128, 1], FP)
            tnr = sb.tile([128, 1], FP)
            ya = sb.tile([128, 1], FP)
            yb = sb.tile([128, 1], FP)
            t2 = sb.tile([128, 1], FP)
            f2 = sb.tile([128, 1], FP)
            halfx = sb.tile([128, 1], FP)
            c15 = sb.tile([128, 1], FP)
            zb = sb.tile([128, 1], FP)
            ident = sb.tile([128, 128], FP)
            diagKI = sb.tile([128, 128], FP)
            W1Tn = sb.tile([64, 128], FP)
            keysno = sb.tile([128, 4 * 65], FP)
            hno = sb.tile([128, 4 * 129], FP)
            rbdw = sb.tile([128, 512], FP)
            bdz = sb.tile([128, 1024], FP)
            lsq = sb.tile([128, 256], FP)
            losspart = sb.tile([128, 4], FP)
            w2pack = sb.tile([128, 129], FP)
            w1pack = sb.tile([65, 256], FP)
            t1 = sb.tile([64, 129], FP)
            tw1 = sb.tile([65, 128], FP)

            gw2 = ps.tile([128, 129], FP, tag="gw2")
            gw1 = ps.tile([65, 256], FP, tag="gw1")

            nc.sync.dma_start(blob[:, 0:C_W2T], blob_d[:, 0:C_W2T])
            nc.sync.dma_start(blob[:, C_W2T:NCOL], blob_d[:, C_W2T:NCOL])

            masks.make_identity(nc, ident[:])
            nc.gpsimd.memset(zb[:], 0.0)
            nc.gpsimd.memset(c15[:], 1.5)
            nc.gpsimd.memset(keysno[:], 1.0)
            nc.gpsimd.memset(hno[:], 1.0)

            v = nc.vector
            g = nc.gpsimd

            if stage >= 2:
                # HW constraint (found empirically): a PSUM bank must only be
                # written by matmuls of a single tile_position, so the two
                # halves (positions (0,0) and (64,0)) go to separate banks.
                zkva = ps.tile([128, 256], FP, tag="zkva", bufs=1)
                zkvb = ps.tile([128, 256], FP, tag="zkvb", bufs=1)
                nc.tensor.matmul(zkva[:], blob[0:64, C_WKV:C_W2T],
                                 blob[0:64, C_XT:C_XT + 256],
                                 start=True, stop=True)
                nc.tensor.matmul(zkvb[:], blob[64:128, C_WKV:C_W2T],
                                 blob[64:128, C_XT:C_XT + 256],
                                 start=True, stop=True)

                # ACT order keeps exactly two table loads: all Silu first, one
                # Derivative_silu, then Copy/Square (present in every table).
                nc.scalar.activation(TK[:, 0:256], zkva[:], ACTF.Silu,
                                     bias=zb[:])
                nc.scalar.activation(TK[:, 256:512], zkvb[:], ACTF.Silu,
                                     bias=zb[:])
                nc.scalar.activation(sqs[:], TK[:], ACTF.Square, bias=zb[:],
                                     accum_out=ssq[:])

            if stage >= 3:
                # rsqrt(ssq) via bit-trick seed + 3 Newton iterations (DVE
                # only; Sqrt ACT funcs live in a third table -> extra load)
                v.tensor_scalar(tnr[:].bitcast(I32), ssq[:].bitcast(I32), 1,
                                None, ALU.logical_shift_right)
                v.tensor_scalar(ya[:].bitcast(I32), tnr[:].bitcast(I32), MAGIC,
                                -1.0, ALU.subtract, ALU.mult)
                v.tensor_scalar(halfx[:], ssq[:], -0.5, None, ALU.mult)
                y = ya
                for _ in range(3):
                    yn = yb if y is ya else ya
                    v.tensor_tensor(t2[:], y[:], y[:], ALU.mult)
                    v.scalar_tensor_tensor(f2[:], t2[:], halfx[:], c15[:],
                                           ALU.mult, ALU.add)
                    v.tensor_tensor(yn[:], f2[:], y[:], ALU.mult)
                    y = yn
                rn = y

                g.tensor_scalar(diagKI[0:64, :], ident[0:64, :], rn[0:64, :],
                                None, ALU.mult)
                g.tensor_copy(diagKI[64:128, :], ident[64:128, :])
                g.tensor_scalar(W1Tn[:], blob[0:64, C_W1T:C_W1T + 128],
                                rn[0:64, :], None, ALU.mult)
                g.tensor_scalar(vT[64:128, :], TK[64:128, :], rn[64:128, :],
                                None, ALU.mult)

            if stage >= 4:
                z1 = ps.tile([128, T], FP, tag="z1", bufs=1)
                nc.tensor.matmul(z1[:], W1Tn[:], TK[0:64, :],
                                 start=True, stop=True)
                nc.scalar.activation(HT[:], z1[:], ACTF.Silu,
                                     bias=blob[:, C_B1:C_B1 + 1])
                nc.scalar.activation(dsl[:], z1[:], ACTF.Derivative_silu,
                                     bias=blob[:, C_B1:C_B1 + 1])

            if stage >= 5:
                yps = ps.tile([128, T], FP, tag="yps", bufs=1)
                nc.tensor.matmul(yps[64:128, :], blob[:, C_W2T:C_W2T + 64],
                                 HT[:], start=True, stop=True)
                # residual R = (y + b2) - v_norm, overwrites value rows of TK
                v.scalar_tensor_tensor(TK[64:128, :], yps[64:128, :],
                                       blob[64:128, C_B2:C_B2 + 1],
                                       vT[64:128, :], ALU.add, ALU.subtract)

                dh = ps.tile([128, T], FP, tag="dh", bufs=1)
                nc.tensor.matmul(dh[:], blob[64:128, C_W2N:C_W2N + 128],
                                 TK[64:128, :], start=True, stop=True)
                v.tensor_tensor(dzT[:], dh[:], dsl[:], ALU.mult)

            if stage >= 6:
                kdh = ps.tile([128, 384], FP, tag="kdh", bufs=1)
            for j in range(4 if stage >= 6 else 0):
                c0 = j * 128
                nc.tensor.matmul(kdh[:, 0:128], TK[:, c0:c0 + 128], diagKI[:],
                                 start=True, stop=True, skip_group_check=True)
                nc.tensor.matmul(kdh[:, 128:256], dzT[:, c0:c0 + 128],
                                 ident[:], is_transpose=True, start=True,
                                 stop=True, skip_group_check=True)
                nc.tensor.matmul(kdh[:, 256:384], HT[:, c0:c0 + 128],
                                 ident[:], is_transpose=True, start=True,
                                 stop=True, skip_group_check=True)

                nc.scalar.activation(keysno[:, 65 * j:65 * j + 64],
                                     kdh[:, 0:64], ACTF.Copy, bias=0.0)
                nc.scalar.activation(hno[:, 129 * j:129 * j + 128],
                                     kdh[:, 256:384], ACTF.Copy, bias=0.0)
                nc.scalar.activation(lsq[:, 64 * j:64 * j + 64],
                                     kdh[:, 64:128], ACTF.Square, bias=zb[:],
                                     accum_out=losspart[:, j:j + 1])

                v.tensor_scalar(rbdw[:, 128 * j:128 * j + 64], kdh[:, 64:128],
                                blob[:, C_BC + j:C_BC + j + 1], None, ALU.mult)
                v.tensor_scalar(rbdw[:, 128 * j + 64:128 * j + 128],
                                kdh[:, 64:128],
                                blob[:, C_DC + j:C_DC + j + 1], None, ALU.mult)
                v.tensor_scalar(bdz[:, 256 * j:256 * j + 128],
                                kdh[:, 128:256],
                                blob[:, C_BC + j:C_BC + j + 1], None, ALU.mult)
                v.tensor_scalar(bdz[:, 256 * j + 128:256 * j + 256],
                                kdh[:, 128:256],
                                blob[:, C_DC + j:C_DC + j + 1], None, ALU.mult)

                nc.tensor.matmul(gw2[:], rbdw[:, 128 * j:128 * j + 128],
                                 hno[:, 129 * j:129 * j + 129],
                                 start=(j == 0), stop=(j == 3),
                                 skip_group_check=True)
                nc.tensor.matmul(gw1[:], keysno[:, 65 * j:65 * j + 65],
                                 bdz[:, 256 * j:256 * j + 256],
                                 start=(j == 0), stop=(j == 3),
                                 skip_group_check=True)

            if stage >= 7:
                v.scalar_tensor_tensor(t1[:], blob[0:64, C_MW2:C_MW2 + 129],
                                       p_T, gw2[0:64, :], ALU.mult, ALU.add)
                v.scalar_tensor_tensor(w2pack[0:64, :],
                                       blob[0:64, C_SW2A:C_SW2A + 129], A_T,
                                       t1[:], ALU.mult, ALU.add)
                v.scalar_tensor_tensor(w2pack[64:128, :],
                                       blob[64:128, C_SW2B:C_SW2B + 129], q_T,
                                       gw2[64:128, :], ALU.mult, ALU.add)
                v.scalar_tensor_tensor(tw1[:], blob[0:65, C_W1M:C_W1M + 128],
                                       p_T, gw1[:, 0:128], ALU.mult, ALU.add)
                v.scalar_tensor_tensor(w1pack[:, 0:128],
                                       blob[0:65, C_W1S:C_W1S + 128], A_T,
                                       tw1[:], ALU.mult, ALU.add)
                v.scalar_tensor_tensor(w1pack[:, 128:256],
                                       blob[0:65, C_W1S:C_W1S + 128], q_T,
                                       gw1[:, 128:256], ALU.mult, ALU.add)
            else:
                nc.gpsimd.memset(w2pack[:], 0.0)
                nc.gpsimd.memset(w1pack[:], 0.0)
                nc.gpsimd.memset(losspart[:], 0.0)

            nc.sync.dma_start(w2pack_d[:], w2pack[:])
            nc.sync.dma_start(w1pack_d[:], w1pack[:])
            nc.sync.dma_start(loss_d[:], losspart[:])

    nc.compile()
    return nc


_NC = None


def _get_nc():
    global _NC
    if _NC is None:
        _NC = _build_nc()
    return _NC


def _pack_blobs(x, WK, WV, W1, b1, W2, b2, SW1, Sb1, SW2, Sb2):
    Bc, Dc, _, _, _ = _coeffs()
    WKV = np.concatenate([WK.T, WV.T], axis=1).astype(np.float32)
    Bct = np.ascontiguousarray(Bc.reshape(4, 128).T)
    Dct = np.ascontiguousarray(Dc.reshape(4, 128).T)
    blobs = []
    for b in range(B):
        bl = np.zeros((128, NCOL), np.float32)
        xT = x[b].T
        bl[0:64, 0:256] = xT[:, 0:256]
        bl[64:128, 0:256] = xT[:, 256:512]
        bl[0:64, C_WKV:C_W2T] = WKV
        bl[64:128, C_WKV:C_W2T] = WKV
        bl[:, C_W2T:C_B1] = W2[b].T
        bl[:, C_B1] = b1[b]
        bl[:, C_BC:C_BC + 4] = Bct
        bl[:, C_DC:C_DC + 4] = Dct
        bl[0:64, C_W1T:C_W1T + 128] = W1[b].T
        bl[0:64, C_MW2:C_MW2 + 128] = W2[b]
        bl[0:64, C_MW2 + 128] = b2[b]
        bl[0:64, C_SW2A:C_SW2A + 128] = SW2[b]
        bl[0:64, C_SW2A + 128] = Sb2[b]
        bl[64:128, C_W2N:C_W2N + 128] = W2[b]
        bl[64:128, C_B2] = b2[b]
        bl[64:128, C_SW2B:C_SW2B + 128] = SW2[b]
        bl[64:128, C_SW2B + 128] = Sb2[b]
        bl[0:64, C_W1M:C_W1M + 128] = W1[b].T
        bl[64, C_W1M:C_W1M + 128] = b1[b]
        bl[0:64, C_W1S:C_W1S + 128] = SW1[b].T
        bl[64, C_W1S:C_W1S + 128] = Sb1[b]
        blobs.append(bl)
    return blobs


def _unpack(results):
    losses = np.zeros((128, 4), np.float32)
    M_W1 = np.empty((B, H, E), np.float32)
    M_b1 = np.empty((B, H), np.float32)
    M_W2 = np.empty((B, E, H), np.float32)
    M_b2 = np.empty((B, E), np.float32)
    S_W1 = np.empty((B, H, E), np.float32)
    S_b1 = np.empty((B, H), np.float32)
    S_W2 = np.empty((B, E, H), np.float32)
    S_b2 = np.empty((B, E), np.float32)
    for b in range(B):
        r = results[b]
        w2p = np.asarray(r["w2pack"])
        w1p = np.asarray(r["w1pack"])
        losses += np.asarray(r["loss"])
        M_W2[b] = w2p[0:64, 0:128]
        M_b2[b] = w2p[0:64, 128]
        S_W2[b] = w2p[64:128, 0:128]
        S_b2[b] = w2p[64:128, 128]
        M_W1[b] = w1p[0:64, 0:128].T
        M_b1[b] = w1p[64, 0:128]
        S_W1[b] = w1p[0:64, 128:256].T
        S_b1[b] = w1p[64, 128:256]
    losses_flat = (losses.T.ravel() * np.float32(1.0 / 512.0)).astype(np.float32)
    return np.concatenate([
        losses_flat,
        M_W1.ravel(), M_b1.ravel(), M_W2.ravel(), M_b2.ravel(),
        S_W1.ravel(), S_b1.ravel(), S_W2.ravel(), S_b2.ravel(),
    ]).astype(np.float32)


def kernel(**inputs):
    nc = _get_nc()
    blobs = _pack_blobs(**{k: np.asarray(inputs[k], np.float32) for k in
                           ("x", "WK", "WV", "W1", "b1", "W2", "b2",
                            "SW1", "Sb1", "SW2", "Sb2")})
    in_maps = [{"blob": bl} for bl in blobs]
    res = run_bass_kernel_spmd(nc, in_maps, list(range(B)))
    return _unpack(res.results)


# revision 21
# speedup vs baseline: 9427.5000x; 9427.5000x over previous
import sys

import numpy as np

if "/opt/trn_rl_repo" not in sys.path:
    sys.path.insert(0, "/opt/trn_rl_repo")

import concourse.bass as bass
import concourse.tile as tile
from concourse import bacc, masks, mybir
from concourse.bass_utils import run_bass_kernel_spmd

B, T, E, H = 8, 512, 64, 128
FP = mybir.dt.float32
I32 = mybir.dt.int32
ALU = mybir.AluOpType
ACTF = mybir.ActivationFunctionType

NCOL = 1099
C_XT = 0       # (128, 256) x[b].T folded: rows 0:64 toks 0:256, rows 64:128 toks 256:512
C_WKV = 256    # (128, 128) [WK.T | WV.T], duplicated in both row halves
C_W2T = 384    # (128, 64)  W2[b].T
C_B1 = 448     # (128, 1)   b1[b]
C_BC = 449     # (128, 4)   Bc'[128j+p] at [p, j]
C_DC = 453     # (128, 4)   Dc'
C_W1T = 457    # rows 0:64  (64,128) W1[b].T
C_MW2 = 585    # rows 0:64  (64,129) [W2[b] | b2[b]]
C_SW2A = 714   # rows 0:64  (64,129) [SW2[b] | Sb2[b]]
C_W2N = 457    # rows 64:128 (64,128) W2[b]
C_B2 = 585     # rows 64:128 (64,1)  b2[b]
C_SW2B = 586   # rows 64:128 (64,129) [SW2[b] | Sb2[b]]
C_W1M = 843    # rows 0:65  (65,128) [W1[b].T ; b1[b]]
C_W1S = 971    # rows 0:65  (65,128) [SW1[b].T ; Sb1[b]]

MAGIC = float(0x5F3759DF)


def _coeffs():
    beta = np.float32(1.0 - 0.999)
    eta = np.float32(0.95)
    theta = np.float32(0.05)
    beta_vec = np.full(T, beta, np.float32)
    eta_vec = np.full(T, eta, np.float32)
    p_prefix = np.cumprod(beta_vec, dtype=np.float32)
    p_suffix = np.cumprod(beta_vec[::-1], dtype=np.float32)[::-1]
    q_prefix = np.cumprod(eta_vec, dtype=np.float32)
    q_suffix = np.cumprod(eta_vec[::-1], dtype=np.float32)[::-1]
    p_T = np.float32(p_prefix[-1])
    q_T = np.float32(q_prefix[-1])
    w = (p_suffix / beta_vec * q_prefix).astype(np.float32)
    A_T = np.float32(w.sum(dtype=np.float32))
    partial = np.cumsum(w[::-1], dtype=np.float32)[::-1].astype(np.float32)
    Bco = (-theta * partial / q_prefix).astype(np.float32)
    Dco = (-theta * q_suffix).astype(np.float32)
    s = np.float32(2.0 / 512.0)  # grad-of-mean factor, exact power of two
    return (Bco * s).astype(np.float32), (Dco * s).astype(np.float32), \
        float(p_T), float(A_T), float(q_T)


# float32r matmuls fault on this runtime (verified via minimal repro);
# keep fp32 — fr is a hook left as identity.
def fr(ap):
    return ap


def _build_nc(stage=7):
    _, _, p_T, A_T, q_T = _coeffs()
    nc = bacc.Bacc(None, target_bir_lowering=False, debug=False)

    blob_d = nc.declare_dram_parameter("blob", [128, NCOL], FP, isOutput=False)
    w2pack_d = nc.declare_dram_parameter("w2pack", [128, 129], FP, isOutput=True)
    w1pack_d = nc.declare_dram_parameter("w1pack", [65, 256], FP, isOutput=True)
    loss_d = nc.declare_dram_parameter("loss", [128, 4], FP, isOutput=True)

    with tile.TileContext(nc) as tc:
        with (
            tc.tile_pool(name="sb", bufs=1) as sb,
            tc.tile_pool(name="ps", bufs=1, space=bass.MemorySpace.PSUM) as ps,
        ):
            blob = sb.tile([128, NCOL], FP)
            TK = sb.tile([128, T], FP)
            vT = sb.tile([128, T], FP)
            HT = sb.tile([128, T], FP)
            dsl = sb.tile([128, T], FP)
            dzT = sb.tile([128, T], FP)
            sqs = sb.tile([128, T], FP)
            ssq = sb.tile([128, 1], FP)
            tnr = sb.tile([128, 1], FP)
            ya = sb.tile([128, 1], FP)
            yb = sb.tile([128, 1], FP)
            t2 = sb.tile([128, 1], FP)
            f2 = sb.tile([128, 1], FP)
            halfx = sb.tile([128, 1], FP)
            c15 = sb.tile([128, 1], FP)
            zb = sb.tile([128, 1], FP)
            ident = sb.tile([128, 128], FP)
            diagKI = sb.tile([128, 128], FP)
            W1Tn = sb.tile([64, 128], FP)
            keysno = sb.tile([128, 4 * 65], FP)
            hno = sb.tile([128, 4 * 129], FP)
            rbdw = sb.tile([128, 512], FP)
            bdz = sb.tile([128, 1024], FP)
            lsq = sb.tile([128, 256], FP)
            losspart = sb.tile([128, 4], FP)
            w2pack = sb.tile([128, 129], FP)
            w1pack = sb.tile([65, 256], FP)
            t1 = sb.tile([64, 129], FP)
            tw1 = sb.tile([65, 128], FP)

            gw2 = ps.tile([128, 129], FP, tag="gw2")
            gw1 = ps.tile([65, 256], FP, tag="gw1")

            nc.sync.dma_start(blob[:, 0:C_W2T], blob_d[:, 0:C_W2T])
            nc.sync.dma_start(blob[:, C_W2T:NCOL], blob_d[:, C_W2T:NCOL])

            masks.make_identity(nc, ident[:])
            nc.vector.memset(zb[:], 0.0)
            nc.vector.memset(c15[:], 1.5)
            nc.vector.memset(keysno[:], 1.0)
            nc.vector.memset(hno[:], 1.0)

            v = nc.vector
            g = nc.gpsimd

            if stage >= 2:
                # HW constraint (found empirically): a PSUM bank must only be
                # written by matmuls of a single tile_position, so the two
                # halves (positions (0,0) and (64,0)) go to separate banks.
                zkva = ps.tile([128, 256], FP, tag="zkva", bufs=1)
                zkvb = ps.tile([128, 256], FP, tag="zkvb", bufs=1)
                nc.tensor.matmul(zkva[:], fr(blob[0:64, C_WKV:C_W2T]),
                                 fr(blob[0:64, C_XT:C_XT + 256]),
                                 start=True, stop=True)
                nc.tensor.matmul(zkvb[:], fr(blob[64:128, C_WKV:C_W2T]),
                                 fr(blob[64:128, C_XT:C_XT + 256]),
                                 start=True, stop=True)

                # ACT order keeps exactly two table loads: all Silu first, one
                # Derivative_silu, then Copy/Square (present in every table).
                nc.scalar.activation(TK[:, 0:256], zkva[:], ACTF.Silu,
                                     bias=zb[:])
                nc.scalar.activation(TK[:, 256:512], zkvb[:], ACTF.Silu,
                                     bias=zb[:])
                nc.scalar.activation(sqs[:], TK[:], ACTF.Square, bias=zb[:],
                                     accum_out=ssq[:])

            if stage >= 3:
                # rsqrt(ssq) via bit-trick seed + 3 Newton iterations (DVE
                # only; Sqrt ACT funcs live in a third table -> extra load)
                v.tensor_scalar(tnr[:].bitcast(I32), ssq[:].bitcast(I32), 1,
                                None, ALU.logical_shift_right)
                v.tensor_scalar(ya[:].bitcast(I32), tnr[:].bitcast(I32), MAGIC,
                                -1.0, ALU.subtract, ALU.mult)
                v.tensor_scalar(halfx[:], ssq[:], -0.5, None, ALU.mult)
                y = ya
                for _ in range(3):
                    yn = yb if y is ya else ya
                    v.tensor_tensor(t2[:], y[:], y[:], ALU.mult)
                    v.scalar_tensor_tensor(f2[:], t2[:], halfx[:], c15[:],
                                           ALU.mult, ALU.add)
                    v.tensor_tensor(yn[:], f2[:], y[:], ALU.mult)
                    y = yn
                rn = y

                v.tensor_scalar(diagKI[0:64, :], ident[0:64, :], rn[0:64, :],
                                None, ALU.mult)
                v.tensor_copy(diagKI[64:128, :], ident[64:128, :])
                v.tensor_scalar(W1Tn[:], blob[0:64, C_W1T:C_W1T + 128],
                                rn[0:64, :], None, ALU.mult)
                v.tensor_scalar(vT[64:128, :], TK[64:128, :], rn[64:128, :],
                                None, ALU.mult)

            if stage >= 4:
                z1 = ps.tile([128, T], FP, tag="z1", bufs=1)
                nc.tensor.matmul(z1[:], fr(W1Tn[:]), fr(TK[0:64, :]),
                                 start=True, stop=True)
                nc.scalar.activation(HT[:], z1[:], ACTF.Silu,
                                     bias=blob[:, C_B1:C_B1 + 1])
                nc.scalar.activation(dsl[:], z1[:], ACTF.Derivative_silu,
                                     bias=blob[:, C_B1:C_B1 + 1])

            if stage >= 5:
                yps = ps.tile([128, T], FP, tag="yps", bufs=1)
                nc.tensor.matmul(yps[64:128, :], fr(blob[:, C_W2T:C_W2T + 64]),
                                 fr(HT[:]), start=True, stop=True)
                # residual R = (y + b2) - v_norm, overwrites value rows of TK
                v.scalar_tensor_tensor(TK[64:128, :], yps[64:128, :],
                                       blob[64:128, C_B2:C_B2 + 1],
                                       vT[64:128, :], ALU.add, ALU.subtract)

                dh = ps.tile([128, T], FP, tag="dh", bufs=1)
                nc.tensor.matmul(dh[:], fr(blob[64:128, C_W2N:C_W2N + 128]),
                                 fr(TK[64:128, :]), start=True, stop=True)
                v.tensor_tensor(dzT[:], dh[:], dsl[:], ALU.mult)

            if stage >= 6:
                kdh = ps.tile([128, 384], FP, tag="kdh", bufs=1)
            for j in range(4 if stage >= 6 else 0):
                c0 = j * 128
                nc.tensor.matmul(kdh[:, 0:128], fr(TK[:, c0:c0 + 128]),
                                 fr(diagKI[:]),
                                 start=True, stop=True, skip_group_check=True)
                nc.tensor.matmul(kdh[:, 128:256], dzT[:, c0:c0 + 128],
                                 ident[:], is_transpose=True, start=True,
                                 stop=True, skip_group_check=True)
                nc.tensor.matmul(kdh[:, 256:384], HT[:, c0:c0 + 128],
                                 ident[:], is_transpose=True, start=True,
                                 stop=True, skip_group_check=True)

                nc.scalar.activation(keysno[:, 65 * j:65 * j + 64],
                                     kdh[:, 0:64], ACTF.Copy, bias=0.0)
                nc.scalar.activation(hno[:, 129 * j:129 * j + 128],
                                     kdh[:, 256:384], ACTF.Copy, bias=0.0)
                nc.scalar.activation(lsq[:, 64 * j:64 * j + 64],
                                     kdh[:, 64:128], ACTF.Square, bias=zb[:],
                                     accum_out=losspart[:, j:j + 1])

                v.tensor_scalar(rbdw[:, 128 * j:128 * j + 64], kdh[:, 64:128],
                                blob[:, C_BC + j:C_BC + j + 1], None, ALU.mult)
                v.tensor_scalar(rbdw[:, 128 * j + 64:128 * j + 128],
                                kdh[:, 64:128],
                                blob[:, C_DC + j:C_DC + j + 1], None, ALU.mult)
                v.tensor_scalar(bdz[:, 256 * j:256 * j + 128],
                                kdh[:, 128:256],
                                blob[:, C_BC + j:C_BC + j + 1], None, ALU.mult)
                v.tensor_scalar(bdz[:, 256 * j + 128:256 * j + 256],
                                kdh[:, 128:256],
                                blob[:, C_DC + j:C_DC + j + 1], None, ALU.mult)

                nc.tensor.matmul(gw2[:], fr(rbdw[:, 128 * j:128 * j + 128]),
                                 fr(hno[:, 129 * j:129 * j + 129]),
                                 start=(j == 0), stop=(j == 3),
                                 skip_group_check=True)
                nc.tensor.matmul(gw1[:], fr(keysno[:, 65 * j:65 * j + 65]),
                                 fr(bdz[:, 256 * j:256 * j + 256]),
                                 start=(j == 0), stop=(j == 3),
                                 skip_group_check=True)

            if stage >= 7:
                v.scalar_tensor_tensor(t1[:], blob[0:64, C_MW2:C_MW2 + 129],
                                       p_T, gw2[0:64, :], ALU.mult, ALU.add)
                v.scalar_tensor_tensor(w2pack[0:64, :],
                                       blob[0:64, C_SW2A:C_SW2A + 129], A_T,
                                       t1[:], ALU.mult, ALU.add)
                v.scalar_tensor_tensor(w2pack[64:128, :],
                                       blob[64:128, C_SW2B:C_SW2B + 129], q_T,
                                       gw2[64:128, :], ALU.mult, ALU.add)
                v.scalar_tensor_tensor(tw1[:], blob[0:65, C_W1M:C_W1M + 128],
                                       p_T, gw1[:, 0:128], ALU.mult, ALU.add)
                v.scalar_tensor_tensor(w1pack[:, 0:128],
                                       blob[0:65, C_W1S:C_W1S + 128], A_T,
                                       tw1[:], ALU.mult, ALU.add)
                v.scalar_tensor_tensor(w1pack[:, 128:256],
                                       blob[0:65, C_W1S:C_W1S + 128], q_T,
                                       gw1[:, 128:256], ALU.mult, ALU.add)
            else:
                nc.gpsimd.memset(w2pack[:], 0.0)
                nc.gpsimd.memset(w1pack[:], 0.0)
                nc.gpsimd.memset(losspart[:], 0.0)

            nc.sync.dma_start(w2pack_d[:], w2pack[:])
            nc.sync.dma_start(w1pack_d[:], w1pack[:])
            nc.sync.dma_start(loss_d[:], losspart[:])

    nc.compile()
    return nc


_NC = None


def _get_nc():
    global _NC
    if _NC is None:
        _NC = _build_nc()
    return _NC


def _pack_blobs(x, WK, WV, W1, b1, W2, b2, SW1, Sb1, SW2, Sb2):
    Bc, Dc, _, _, _ = _coeffs()
    WKV = np.concatenate([WK.T, WV.T], axis=1).astype(np.float32)
    Bct = np.ascontiguousarray(Bc.reshape(4, 128).T)
    Dct = np.ascontiguousarray(Dc.reshape(4, 128).T)
    blobs = []
    for b in range(B):
        bl = np.zeros((128, NCOL), np.float32)
        xT = x[b].T
        bl[0:64, 0:256] = xT[:, 0:256]
        bl[64:128, 0:256] = xT[:, 256:512]
        bl[0:64, C_WKV:C_W2T] = WKV
        bl[64:128, C_WKV:C_W2T] = WKV
        bl[:, C_W2T:C_B1] = W2[b].T
        bl[:, C_B1] = b1[b]
        bl[:, C_BC:C_BC + 4] = Bct
        bl[:, C_DC:C_DC + 4] = Dct
        bl[0:64, C_W1T:C_W1T + 128] = W1[b].T
        bl[0:64, C_MW2:C_MW2 + 128] = W2[b]
        bl[0:64, C_MW2 + 128] = b2[b]
        bl[0:64, C_SW2A:C_SW2A + 128] = SW2[b]
        bl[0:64, C_SW2A + 128] = Sb2[b]
        bl[64:128, C_W2N:C_W2N + 128] = W2[b]
        bl[64:128, C_B2] = b2[b]
        bl[64:128, C_SW2B:C_SW2B + 128] = SW2[b]
        bl[64:128, C_SW2B + 128] = Sb2[b]
        bl[0:64, C_W1M:C_W1M + 128] = W1[b].T
        bl[64, C_W1M:C_W1M + 128] = b1[b]
        bl[0:64, C_W1S:C_W1S + 128] = SW1[b].T
        bl[64, C_W1S:C_W1S + 128] = Sb1[b]
        blobs.append(bl)
    return blobs


def _unpack(results):
    losses = np.zeros((128, 4), np.float32)
    M_W1 = np.empty((B, H, E), np.float32)
    M_b1 = np.empty((B, H), np.float32)
    M_W2 = np.empty((B, E, H), np.float32)
    M_b2 = np.empty((B, E), np.float32)
    S_W1 = np.empty((B, H, E), np.float32)
    S_b1 = np.empty((B, H), np.float32)
    S_W2 = np.empty((B, E, H), np.float32)
    S_b2 = np.empty((B, E), np.float32)
    for b in range(B):
        r = results[b]
        w2p = np.asarray(r["w2pack"])
        w1p = np.asarray(r["w1pack"])
        losses += np.asarray(r["loss"])
        M_W2[b] = w2p[0:64, 0:128]
        M_b2[b] = w2p[0:64, 128]
        S_W2[b] = w2p[64:128, 0:128]
        S_b2[b] = w2p[64:128, 128]
        M_W1[b] = w1p[0:64, 0:128].T
        M_b1[b] = w1p[64, 0:128]
        S_W1[b] = w1p[0:64, 128:256].T
        S_b1[b] = w1p[64, 128:256]
    losses_flat = (losses.T.ravel() * np.float32(1.0 / 512.0)).astype(np.float32)
    return np.concatenate([
        losses_flat,
        M_W1.ravel(), M_b1.ravel(), M_W2.ravel(), M_b2.ravel(),
        S_W1.ravel(), S_b1.ravel(), S_W2.ravel(), S_b2.ravel(),
    ]).astype(np.float32)


def kernel(**inputs):
    nc = _get_nc()
    blobs = _pack_blobs(**{k: np.asarray(inputs[k], np.float32) for k in
                           ("x", "WK", "WV", "W1", "b1", "W2", "b2",
                            "SW1", "Sb1", "SW2", "Sb2")})
    in_maps = [{"blob": bl} for bl in blobs]
    res = run_bass_kernel_spmd(nc, in_maps, list(range(B)))
    return _unpack(res.results)


# revision 24
# speedup vs baseline: 11018.7028x; 1.1688x over previous
import sys

import numpy as np

if "/opt/trn_rl_repo" not in sys.path:
    sys.path.insert(0, "/opt/trn_rl_repo")

import concourse.bass as bass
import concourse.tile as tile
from concourse import bacc, masks, mybir
from concourse.bass_utils import run_bass_kernel_spmd

B, T, E, H = 8, 512, 64, 128
FP = mybir.dt.float32
I32 = mybir.dt.int32
ALU = mybir.AluOpType
ACTF = mybir.ActivationFunctionType

NCOL = 1099
C_XT = 0       # (128, 256) x[b].T folded: rows 0:64 toks 0:256, rows 64:128 toks 256:512
C_WKV = 256    # (128, 128) [WK.T | WV.T], duplicated in both row halves
C_W2T = 384    # (128, 64)  W2[b].T
C_B1 = 448     # (128, 1)   b1[b]
C_BC = 449     # (128, 4)   Bc'[128j+p] at [p, j]
C_DC = 453     # (128, 4)   Dc'
C_W1T = 457    # rows 0:64  (64,128) W1[b].T
C_MW2 = 585    # rows 0:64  (64,129) [W2[b] | b2[b]]
C_SW2A = 714   # rows 0:64  (64,129) [SW2[b] | Sb2[b]]
C_W2N = 457    # rows 64:128 (64,128) W2[b]
C_B2 = 585     # rows 64:128 (64,1)  b2[b]
C_SW2B = 586   # rows 64:128 (64,129) [SW2[b] | Sb2[b]]
C_W1M = 843    # rows 0:65  (65,128) [W1[b].T ; b1[b]]
C_W1S = 971    # rows 0:65  (65,128) [SW1[b].T ; Sb1[b]]

MAGIC = float(0x5F3759DF)


def _coeffs():
    beta = np.float32(1.0 - 0.999)
    eta = np.float32(0.95)
    theta = np.float32(0.05)
    beta_vec = np.full(T, beta, np.float32)
    eta_vec = np.full(T, eta, np.float32)
    p_prefix = np.cumprod(beta_vec, dtype=np.float32)
    p_suffix = np.cumprod(beta_vec[::-1], dtype=np.float32)[::-1]
    q_prefix = np.cumprod(eta_vec, dtype=np.float32)
    q_suffix = np.cumprod(eta_vec[::-1], dtype=np.float32)[::-1]
    p_T = np.float32(p_prefix[-1])
    q_T = np.float32(q_prefix[-1])
    w = (p_suffix / beta_vec * q_prefix).astype(np.float32)
    A_T = np.float32(w.sum(dtype=np.float32))
    partial = np.cumsum(w[::-1], dtype=np.float32)[::-1].astype(np.float32)
    Bco = (-theta * partial / q_prefix).astype(np.float32)
    Dco = (-theta * q_suffix).astype(np.float32)
    s = np.float32(2.0 / 512.0)  # grad-of-mean factor, exact power of two
    return (Bco * s).astype(np.float32), (Dco * s).astype(np.float32), \
        float(p_T), float(A_T), float(q_T)


# float32r matmuls fault on this runtime (verified via minimal repro);
# keep fp32 — fr is a hook left as identity.
def fr(ap):
    return ap


def _build_nc(stage=7):
    _, _, p_T, A_T, q_T = _coeffs()
    nc = bacc.Bacc(None, target_bir_lowering=False, debug=False)

    blob_d = nc.declare_dram_parameter("blob", [128, NCOL], FP, isOutput=False)
    w2pack_d = nc.declare_dram_parameter("w2pack", [128, 129], FP, isOutput=True)
    w1pack_d = nc.declare_dram_parameter("w1pack", [65, 256], FP, isOutput=True)
    loss_d = nc.declare_dram_parameter("loss", [128, 4], FP, isOutput=True)

    with tile.TileContext(nc) as tc:
        with (
            tc.tile_pool(name="sb", bufs=1) as sb,
            tc.tile_pool(name="ps", bufs=1, space=bass.MemorySpace.PSUM) as ps,
        ):
            blob = sb.tile([128, NCOL], FP)
            TK = sb.tile([128, T], FP)
            vT = sb.tile([128, T], FP)
            HT = sb.tile([128, T], FP)
            dsl = sb.tile([128, T], FP)
            dzT = sb.tile([128, T], FP)
            sqs = sb.tile([128, T], FP)
            ssq = sb.tile([128, 1], FP)
            tnr = sb.tile([128, 1], FP)
            ya = sb.tile([128, 1], FP)
            yb = sb.tile([128, 1], FP)
            t2 = sb.tile([128, 1], FP)
            f2 = sb.tile([128, 1], FP)
            halfx = sb.tile([128, 1], FP)
            c15 = sb.tile([128, 1], FP)
            zb = sb.tile([128, 1], FP)
            ident = sb.tile([128, 128], FP)
            diagKI = sb.tile([128, 128], FP)
            W1Tn = sb.tile([64, 128], FP)
            keysno = sb.tile([128, 4 * 65], FP)
            hno = sb.tile([128, 4 * 129], FP)
            rbdw = sb.tile([128, 512], FP)
            bdz = sb.tile([128, 1024], FP)
            lsq = sb.tile([128, 256], FP)
            losspart = sb.tile([128, 4], FP)
            w2pack = sb.tile([128, 129], FP)
            w1pack = sb.tile([65, 256], FP)
            t1 = sb.tile([64, 129], FP)
            tw1 = sb.tile([65, 128], FP)

            gw2 = ps.tile([128, 129], FP, tag="gw2")
            gw1 = ps.tile([65, 256], FP, tag="gw1")

            # Row-split the first region: zkva only needs rows 0:64, so it
            # can start while rows 64:128 are still in flight.
            nc.sync.dma_start(blob[0:64, 0:C_W2T], blob_d[0:64, 0:C_W2T])
            nc.sync.dma_start(blob[64:128, 0:C_W2T], blob_d[64:128, 0:C_W2T])
            nc.sync.dma_start(blob[:, C_W2T:NCOL], blob_d[:, C_W2T:NCOL])

            masks.make_identity(nc, ident[:])
            nc.vector.memset(zb[:], 0.0)
            nc.vector.memset(c15[:], 1.5)
            nc.vector.memset(keysno[:], 1.0)
            nc.vector.memset(hno[:], 1.0)

            v = nc.vector
            g = nc.gpsimd

            if stage >= 2:
                # HW constraint (found empirically): a PSUM bank must only be
                # written by matmuls of a single tile_position, so the two
                # halves (positions (0,0) and (64,0)) go to separate banks.
                zkva = ps.tile([128, 256], FP, tag="zkva", bufs=1)
                zkvb = ps.tile([128, 256], FP, tag="zkvb", bufs=1)
                nc.tensor.matmul(zkva[:], fr(blob[0:64, C_WKV:C_W2T]),
                                 fr(blob[0:64, C_XT:C_XT + 256]),
                                 start=True, stop=True)
                nc.tensor.matmul(zkvb[:], fr(blob[64:128, C_WKV:C_W2T]),
                                 fr(blob[64:128, C_XT:C_XT + 256]),
                                 start=True, stop=True)

                # ACT order keeps exactly two table loads: all Silu first, one
                # Derivative_silu, then Copy/Square (present in every table).
                nc.scalar.activation(TK[:, 0:256], zkva[:], ACTF.Silu,
                                     bias=zb[:])
                nc.scalar.activation(TK[:, 256:512], zkvb[:], ACTF.Silu,
                                     bias=zb[:])
                nc.scalar.activation(sqs[:], TK[:], ACTF.Square, bias=zb[:],
                                     accum_out=ssq[:])

            if stage >= 3:
                # rsqrt(ssq) via bit-trick seed + 3 Newton iterations (DVE
                # only; Sqrt ACT funcs live in a third table -> extra load)
                v.tensor_scalar(tnr[:].bitcast(I32), ssq[:].bitcast(I32), 1,
                                None, ALU.logical_shift_right)
                v.tensor_scalar(ya[:].bitcast(I32), tnr[:].bitcast(I32), MAGIC,
                                -1.0, ALU.subtract, ALU.mult)
                v.tensor_scalar(halfx[:], ssq[:], -0.5, None, ALU.mult)
                y = ya
                for _ in range(3):
                    yn = yb if y is ya else ya
                    v.tensor_tensor(t2[:], y[:], y[:], ALU.mult)
                    v.scalar_tensor_tensor(f2[:], t2[:], halfx[:], c15[:],
                                           ALU.mult, ALU.add)
                    v.tensor_tensor(yn[:], f2[:], y[:], ALU.mult)
                    y = yn
                rn = y

                # W1Tn first: it gates the z1 matmul; the other scales can
                # run while the PE is busy.
                v.tensor_scalar(W1Tn[:], blob[0:64, C_W1T:C_W1T + 128],
                                rn[0:64, :], None, ALU.mult)
                v.tensor_scalar(diagKI[0:64, :], ident[0:64, :], rn[0:64, :],
                                None, ALU.mult)
                v.tensor_copy(diagKI[64:128, :], ident[64:128, :])
                v.tensor_scalar(vT[64:128, :], TK[64:128, :], rn[64:128, :],
                                None, ALU.mult)

            if stage >= 4:
                z1 = ps.tile([128, T], FP, tag="z1", bufs=1)
                nc.tensor.matmul(z1[:], fr(W1Tn[:]), fr(TK[0:64, :]),
                                 start=True, stop=True)
                nc.scalar.activation(HT[:], z1[:], ACTF.Silu,
                                     bias=blob[:, C_B1:C_B1 + 1])
                nc.scalar.activation(dsl[:], z1[:], ACTF.Derivative_silu,
                                     bias=blob[:, C_B1:C_B1 + 1])

            if stage >= 5:
                yps = ps.tile([128, T], FP, tag="yps", bufs=1)
                nc.tensor.matmul(yps[64:128, :], fr(blob[:, C_W2T:C_W2T + 64]),
                                 fr(HT[:]), start=True, stop=True)
                # residual R = (y + b2) - v_norm, overwrites value rows of TK
                v.scalar_tensor_tensor(TK[64:128, :], yps[64:128, :],
                                       blob[64:128, C_B2:C_B2 + 1],
                                       vT[64:128, :], ALU.add, ALU.subtract)

                dh = ps.tile([128, T], FP, tag="dh", bufs=1)
                nc.tensor.matmul(dh[:], fr(blob[64:128, C_W2N:C_W2N + 128]),
                                 fr(TK[64:128, :]), start=True, stop=True)
                v.tensor_tensor(dzT[:], dh[:], dsl[:], ALU.mult)

            if stage >= 6:
                # Software pipeline over the 4 token chunks. kdh double-
                # buffers between its own bank and a slice of the (dead) z1
                # bank — both banks are written only by tile-position (0,0)
                # matmuls, keeping the single-position-per-bank rule.
                kdh = ps.tile([128, 384], FP, tag="kdh", bufs=1)
                kbufs = [kdh, z1[:, 0:384]]

                def transposes(j):
                    kb = kbufs[j % 2]
                    c0 = j * 128
                    nc.tensor.matmul(kb[:, 0:128], fr(TK[:, c0:c0 + 128]),
                                     fr(diagKI[:]), start=True, stop=True,
                                     skip_group_check=True)
                    nc.tensor.matmul(kb[:, 128:256], dzT[:, c0:c0 + 128],
                                     ident[:], is_transpose=True, start=True,
                                     stop=True, skip_group_check=True)
                    nc.tensor.matmul(kb[:, 256:384], HT[:, c0:c0 + 128],
                                     ident[:], is_transpose=True, start=True,
                                     stop=True, skip_group_check=True)

                def consume(j):
                    kb = kbufs[j % 2]
                    nc.scalar.activation(keysno[:, 65 * j:65 * j + 64],
                                         kb[:, 0:64], ACTF.Copy, bias=0.0)
                    nc.scalar.activation(hno[:, 129 * j:129 * j + 128],
                                         kb[:, 256:384], ACTF.Copy, bias=0.0)
                    nc.scalar.activation(lsq[:, 64 * j:64 * j + 64],
                                         kb[:, 64:128], ACTF.Square,
                                         bias=zb[:],
                                         accum_out=losspart[:, j:j + 1])

                    v.tensor_scalar(rbdw[:, 128 * j:128 * j + 64],
                                    kb[:, 64:128],
                                    blob[:, C_BC + j:C_BC + j + 1], None,
                                    ALU.mult)
                    v.tensor_scalar(rbdw[:, 128 * j + 64:128 * j + 128],
                                    kb[:, 64:128],
                                    blob[:, C_DC + j:C_DC + j + 1], None,
                                    ALU.mult)
                    v.tensor_scalar(bdz[:, 256 * j:256 * j + 128],
                                    kb[:, 128:256],
                                    blob[:, C_BC + j:C_BC + j + 1], None,
                                    ALU.mult)
                    v.tensor_scalar(bdz[:, 256 * j + 128:256 * j + 256],
                                    kb[:, 128:256],
                                    blob[:, C_DC + j:C_DC + j + 1], None,
                                    ALU.mult)

                def gws(j):
                    nc.tensor.matmul(gw2[:],
                                     fr(rbdw[:, 128 * j:128 * j + 128]),
                                     fr(hno[:, 129 * j:129 * j + 129]),
                                     start=(j == 0), stop=(j == 3),
                                     skip_group_check=True)
                    nc.tensor.matmul(gw1[:],
                                     fr(keysno[:, 65 * j:65 * j + 65]),
                                     fr(bdz[:, 256 * j:256 * j + 256]),
                                     start=(j == 0), stop=(j == 3),
                                     skip_group_check=True)

                transposes(0)
                consume(0)
                transposes(1)
                gws(0)
                consume(1)
                transposes(2)
                gws(1)
                consume(2)
                transposes(3)
                gws(2)
                consume(3)
                gws(3)

            if stage >= 7:
                v.scalar_tensor_tensor(t1[:], blob[0:64, C_MW2:C_MW2 + 129],
                                       p_T, gw2[0:64, :], ALU.mult, ALU.add)
                v.scalar_tensor_tensor(w2pack[0:64, :],
                                       blob[0:64, C_SW2A:C_SW2A + 129], A_T,
                                       t1[:], ALU.mult, ALU.add)
                v.scalar_tensor_tensor(w2pack[64:128, :],
                                       blob[64:128, C_SW2B:C_SW2B + 129], q_T,
                                       gw2[64:128, :], ALU.mult, ALU.add)
                v.scalar_tensor_tensor(tw1[:], blob[0:65, C_W1M:C_W1M + 128],
                                       p_T, gw1[:, 0:128], ALU.mult, ALU.add)
                v.scalar_tensor_tensor(w1pack[:, 0:128],
                                       blob[0:65, C_W1S:C_W1S + 128], A_T,
                                       tw1[:], ALU.mult, ALU.add)
                v.scalar_tensor_tensor(w1pack[:, 128:256],
                                       blob[0:65, C_W1S:C_W1S + 128], q_T,
                                       gw1[:, 128:256], ALU.mult, ALU.add)
            else:
                nc.gpsimd.memset(w2pack[:], 0.0)
                nc.gpsimd.memset(w1pack[:], 0.0)
                nc.gpsimd.memset(losspart[:], 0.0)

            nc.sync.dma_start(w2pack_d[:], w2pack[:])
            nc.sync.dma_start(w1pack_d[:], w1pack[:])
            nc.sync.dma_start(loss_d[:], losspart[:])

    nc.compile()
    return nc


_NC = None


def _get_nc():
    global _NC
    if _NC is None:
        _NC = _build_nc()
    return _NC


def _pack_blobs(x, WK, WV, W1, b1, W2, b2, SW1, Sb1, SW2, Sb2):
    Bc, Dc, _, _, _ = _coeffs()
    WKV = np.concatenate([WK.T, WV.T], axis=1).astype(np.float32)
    Bct = np.ascontiguousarray(Bc.reshape(4, 128).T)
    Dct = np.ascontiguousarray(Dc.reshape(4, 128).T)
    blobs = []
    for b in range(B):
        bl = np.zeros((128, NCOL), np.float32)
        xT = x[b].T
        bl[0:64, 0:256] = xT[:, 0:256]
        bl[64:128, 0:256] = xT[:, 256:512]
        bl[0:64, C_WKV:C_W2T] = WKV
        bl[64:128, C_WKV:C_W2T] = WKV
        bl[:, C_W2T:C_B1] = W2[b].T
        bl[:, C_B1] = b1[b]
        bl[:, C_BC:C_BC + 4] = Bct
        bl[:, C_DC:C_DC + 4] = Dct
        bl[0:64, C_W1T:C_W1T + 128] = W1[b].T
        bl[0:64, C_MW2:C_MW2 + 128] = W2[b]
        bl[0:64, C_MW2 + 128] = b2[b]
        bl[0:64, C_SW2A:C_SW2A + 128] = SW2[b]
        bl[0:64, C_SW2A + 128] = Sb2[b]
        bl[64:128, C_W2N:C_W2N + 128] = W2[b]
        bl[64:128, C_B2] = b2[b]
        bl[64:128, C_SW2B:C_SW2B + 128] = SW2[b]
        bl[64:128, C_SW2B + 128] = Sb2[b]
        bl[0:64, C_W1M:C_W1M + 128] = W1[b].T
        bl[64, C_W1M:C_W1M + 128] = b1[b]
        bl[0:64, C_W1S:C_W1S + 128] = SW1[b].T
        bl[64, C_W1S:C_W1S + 128] = Sb1[b]
        blobs.append(bl)
    return blobs


def _unpack(results):
    losses = np.zeros((128, 4), np.float32)
    M_W1 = np.empty((B, H, E), np.float32)
    M_b1 = np.empty((B, H), np.float32)
    M_W2 = np.empty((B, E, H), np.float32)
    M_b2 = np.empty((B, E), np.float32)
    S_W1 = np.empty((B, H, E), np.float32)
    S_b1 = np.empty((B, H), np.float32)
    S_W2 = np.empty((B, E, H), np.float32)
    S_b2 = np.empty((B, E), np.float32)
    for b in range(B):
        r = results[b]
        w2p = np.asarray(r["w2pack"])
        w1p = np.asarray(r["w1pack"])
        losses += np.asarray(r["loss"])
        M_W2[b] = w2p[0:64, 0:128]
        M_b2[b] = w2p[0:64, 128]
        S_W2[b] = w2p[64:128, 0:128]
        S_b2[b] = w2p[64:128, 128]
        M_W1[b] = w1p[0:64, 0:128].T
        M_b1[b] = w1p[64, 0:128]
        S_W1[b] = w1p[0:64, 128:256].T
        S_b1[b] = w1p[64, 128:256]
    losses_flat = (losses.T.ravel() * np.float32(1.0 / 512.0)).astype(np.float32)
    return np.concatenate([
        losses_flat,
        M_W1.ravel(), M_b1.ravel(), M_W2.ravel(), M_b2.ravel(),
        S_W1.ravel(), S_b1.ravel(), S_W2.ravel(), S_b2.ravel(),
    ]).astype(np.float32)


def kernel(**inputs):
    nc = _get_nc()
    blobs = _pack_blobs(**{k: np.asarray(inputs[k], np.float32) for k in
                           ("x", "WK", "WV", "W1", "b1", "W2", "b2",
                            "SW1", "Sb1", "SW2", "Sb2")})
    in_maps = [{"blob": bl} for bl in blobs]
    res = run_bass_kernel_spmd(nc, in_maps, list(range(B)))
    return _unpack(res.results)


# revision 27
# speedup vs baseline: 11148.0278x; 1.0117x over previous
import sys

import numpy as np

if "/opt/trn_rl_repo" not in sys.path:
    sys.path.insert(0, "/opt/trn_rl_repo")

import concourse.bass as bass
import concourse.tile as tile
from concourse import bacc, masks, mybir
from concourse.bass_utils import run_bass_kernel_spmd

B, T, E, H = 8, 512, 64, 128
FP = mybir.dt.float32
I32 = mybir.dt.int32
ALU = mybir.AluOpType
ACTF = mybir.ActivationFunctionType

NCOL = 1099
C_XT = 0       # (128, 256) x[b].T folded: rows 0:64 toks 0:256, rows 64:128 toks 256:512
C_WKV = 256    # (128, 128) [WK.T | WV.T], duplicated in both row halves
C_W2T = 384    # (128, 64)  W2[b].T
C_B1 = 448     # (128, 1)   b1[b]
C_BC = 449     # (128, 4)   Bc'[128j+p] at [p, j]
C_DC = 453     # (128, 4)   Dc'
C_W1T = 457    # rows 0:64  (64,128) W1[b].T
C_MW2 = 585    # rows 0:64  (64,129) [W2[b] | b2[b]]
C_SW2A = 714   # rows 0:64  (64,129) [SW2[b] | Sb2[b]]
C_W2N = 457    # rows 64:128 (64,128) W2[b]
C_B2 = 585     # rows 64:128 (64,1)  b2[b]
C_SW2B = 586   # rows 64:128 (64,129) [SW2[b] | Sb2[b]]
C_W1M = 843    # rows 0:65  (65,128) [W1[b].T ; b1[b]]
C_W1S = 971    # rows 0:65  (65,128) [SW1[b].T ; Sb1[b]]

MAGIC = float(0x5F3759DF)


def _coeffs():
    beta = np.float32(1.0 - 0.999)
    eta = np.float32(0.95)
    theta = np.float32(0.05)
    beta_vec = np.full(T, beta, np.float32)
    eta_vec = np.full(T, eta, np.float32)
    p_prefix = np.cumprod(beta_vec, dtype=np.float32)
    p_suffix = np.cumprod(beta_vec[::-1], dtype=np.float32)[::-1]
    q_prefix = np.cumprod(eta_vec, dtype=np.float32)
    q_suffix = np.cumprod(eta_vec[::-1], dtype=np.float32)[::-1]
    p_T = np.float32(p_prefix[-1])
    q_T = np.float32(q_prefix[-1])
    w = (p_suffix / beta_vec * q_prefix).astype(np.float32)
    A_T = np.float32(w.sum(dtype=np.float32))
    partial = np.cumsum(w[::-1], dtype=np.float32)[::-1].astype(np.float32)
    Bco = (-theta * partial / q_prefix).astype(np.float32)
    Dco = (-theta * q_suffix).astype(np.float32)
    s = np.float32(2.0 / 512.0)  # grad-of-mean factor, exact power of two
    return (Bco * s).astype(np.float32), (Dco * s).astype(np.float32), \
        float(p_T), float(A_T), float(q_T)


# float32r matmuls fault on this runtime (verified via minimal repro);
# keep fp32 — fr is a hook left as identity.
def fr(ap):
    return ap


def _build_nc(stage=7):
    _, _, p_T, A_T, q_T = _coeffs()
    nc = bacc.Bacc(None, target_bir_lowering=False, debug=False)

    blob_d = nc.declare_dram_parameter("blob", [128, NCOL], FP, isOutput=False)
    w2pack_d = nc.declare_dram_parameter("w2pack", [128, 129], FP, isOutput=True)
    w1pack_d = nc.declare_dram_parameter("w1pack", [65, 256], FP, isOutput=True)
    loss_d = nc.declare_dram_parameter("loss", [128, 4], FP, isOutput=True)

    with tile.TileContext(nc) as tc:
        with (
            tc.tile_pool(name="sb", bufs=1) as sb,
            tc.tile_pool(name="ps", bufs=1, space=bass.MemorySpace.PSUM) as ps,
        ):
            blob = sb.tile([128, NCOL], FP)
            TK = sb.tile([128, T], FP)
            vT = sb.tile([128, T], FP)
            HT = sb.tile([128, T], FP)
            dsl = sb.tile([128, T], FP)
            dzT = sb.tile([128, T], FP)
            sqs = sb.tile([128, T], FP)
            ssq = sb.tile([128, 1], FP)
            tnr = sb.tile([128, 1], FP)
            ya = sb.tile([128, 1], FP)
            yb = sb.tile([128, 1], FP)
            t2 = sb.tile([128, 1], FP)
            f2 = sb.tile([128, 1], FP)
            halfx = sb.tile([128, 1], FP)
            c15 = sb.tile([128, 1], FP)
            zb = sb.tile([128, 1], FP)
            ident = sb.tile([128, 128], FP)
            diagKI = sb.tile([128, 128], FP)
            W1Tn = sb.tile([64, 128], FP)
            keysno = sb.tile([128, 4 * 65], FP)
            hno = sb.tile([128, 4 * 129], FP)
            rbdw = sb.tile([128, 512], FP)
            bdz = sb.tile([128, 1024], FP)
            lsq = sb.tile([128, 256], FP)
            losspart = sb.tile([128, 4], FP)
            w2pack = sb.tile([128, 129], FP)
            w1pack = sb.tile([65, 256], FP)
            t1 = sb.tile([64, 129], FP)
            tw1 = sb.tile([65, 128], FP)

            gw2 = ps.tile([128, 129], FP, tag="gw2")
            gw1 = ps.tile([65, 256], FP, tag="gw1")

            nc.sync.dma_start(blob[:, 0:C_W2T], blob_d[:, 0:C_W2T])
            nc.sync.dma_start(blob[:, C_W2T:NCOL], blob_d[:, C_W2T:NCOL])

            masks.make_identity(nc, ident[:])
            nc.vector.memset(zb[:], 0.0)
            nc.vector.memset(c15[:], 1.5)
            nc.vector.memset(keysno[:], 1.0)
            nc.vector.memset(hno[:], 1.0)

            v = nc.vector
            g = nc.gpsimd

            if stage >= 2:
                # HW constraint (found empirically): a PSUM bank must only be
                # written by matmuls of a single tile_position, so the two
                # halves (positions (0,0) and (64,0)) go to separate banks.
                zkva = ps.tile([128, 256], FP, tag="zkva", bufs=1)
                zkvb = ps.tile([128, 256], FP, tag="zkvb", bufs=1)
                nc.tensor.matmul(zkva[:], fr(blob[0:64, C_WKV:C_W2T]),
                                 fr(blob[0:64, C_XT:C_XT + 256]),
                                 start=True, stop=True)
                nc.tensor.matmul(zkvb[:], fr(blob[64:128, C_WKV:C_W2T]),
                                 fr(blob[64:128, C_XT:C_XT + 256]),
                                 start=True, stop=True)

                # ACT order keeps exactly two table loads: all Silu first, one
                # Derivative_silu, then Copy/Square (present in every table).
                nc.scalar.activation(TK[:, 0:256], zkva[:], ACTF.Silu,
                                     bias=zb[:])
                nc.scalar.activation(TK[:, 256:512], zkvb[:], ACTF.Silu,
                                     bias=zb[:])
                nc.scalar.activation(sqs[:], TK[:], ACTF.Square, bias=zb[:],
                                     accum_out=ssq[:])

            if stage >= 3:
                # rsqrt(ssq) via bit-trick seed + 3 Newton iterations (DVE
                # only; Sqrt ACT funcs live in a third table -> extra load)
                v.tensor_scalar(tnr[:].bitcast(I32), ssq[:].bitcast(I32), 1,
                                None, ALU.logical_shift_right)
                v.tensor_scalar(ya[:].bitcast(I32), tnr[:].bitcast(I32), MAGIC,
                                -1.0, ALU.subtract, ALU.mult)
                v.tensor_scalar(halfx[:], ssq[:], -0.5, None, ALU.mult)
                y = ya
                for _ in range(3):
                    yn = yb if y is ya else ya
                    # y' = (y*halfx*y + 1.5)*y, two fused tensor_scalar ops
                    v.tensor_scalar(t2[:], y[:], halfx[:, :], y[:, :],
                                    ALU.mult, ALU.mult)
                    v.tensor_scalar(yn[:], t2[:], 1.5, y[:, :],
                                    ALU.add, ALU.mult)
                    y = yn
                rn = y

                # W1Tn first: it gates the z1 matmul; the other scales can
                # run while the PE is busy.
                v.tensor_scalar(W1Tn[:], blob[0:64, C_W1T:C_W1T + 128],
                                rn[0:64, :], None, ALU.mult)
                v.tensor_scalar(diagKI[0:64, :], ident[0:64, :], rn[0:64, :],
                                None, ALU.mult)
                v.tensor_copy(diagKI[64:128, :], ident[64:128, :])
                v.tensor_scalar(vT[64:128, :], TK[64:128, :], rn[64:128, :],
                                None, ALU.mult)

            if stage >= 4:
                # Forward/backward chain split into token halves so the ACT
                # and DVE steps of half a overlap the PE matmuls of half b.
                z1 = ps.tile([128, T], FP, tag="z1", bufs=1)
                yps = ps.tile([128, T], FP, tag="yps", bufs=1)
                dh = ps.tile([128, T], FP, tag="dh", bufs=1)
                HVS = [(0, 256), (256, 512)]
                for a, b in HVS:
                    nc.tensor.matmul(z1[:, a:b], fr(W1Tn[:]),
                                     fr(TK[0:64, a:b]), start=True, stop=True,
                                     skip_group_check=True)
                if stage >= 5:
                    for a, b in HVS:
                        nc.scalar.activation(HT[:, a:b], z1[:, a:b], ACTF.Silu,
                                             bias=blob[:, C_B1:C_B1 + 1])
                        nc.tensor.matmul(yps[64:128, a:b],
                                         fr(blob[:, C_W2T:C_W2T + 64]),
                                         fr(HT[:, a:b]), start=True, stop=True,
                                         skip_group_check=True)
                    # Both Derivative_silu ops stay contiguous: exactly one
                    # table switch away from Silu, and dzT is consumed late.
                    for a, b in HVS:
                        nc.scalar.activation(dsl[:, a:b], z1[:, a:b],
                                             ACTF.Derivative_silu,
                                             bias=blob[:, C_B1:C_B1 + 1])
                    for a, b in HVS:
                        # residual R = (y + b2) - v_norm, overwrites V rows
                        v.scalar_tensor_tensor(TK[64:128, a:b],
                                               yps[64:128, a:b],
                                               blob[64:128, C_B2:C_B2 + 1],
                                               vT[64:128, a:b], ALU.add,
                                               ALU.subtract)
                        nc.tensor.matmul(dh[:, a:b],
                                         fr(blob[64:128, C_W2N:C_W2N + 128]),
                                         fr(TK[64:128, a:b]), start=True,
                                         stop=True, skip_group_check=True)
                    for a, b in HVS:
                        v.tensor_tensor(dzT[:, a:b], dh[:, a:b], dsl[:, a:b],
                                        ALU.mult)
                else:
                    nc.scalar.activation(HT[:], z1[:], ACTF.Silu,
                                         bias=blob[:, C_B1:C_B1 + 1])
                    nc.scalar.activation(dsl[:], z1[:], ACTF.Derivative_silu,
                                         bias=blob[:, C_B1:C_B1 + 1])

            if stage >= 6:
                # Software pipeline over the 4 token chunks. kdh double-
                # buffers between its own bank and a slice of the (dead) z1
                # bank — both banks are written only by tile-position (0,0)
                # matmuls, keeping the single-position-per-bank rule.
                kdh = ps.tile([128, 384], FP, tag="kdh", bufs=1)
                kbufs = [kdh, z1[:, 0:384]]

                def transposes(j):
                    kb = kbufs[j % 2]
                    c0 = j * 128
                    nc.tensor.matmul(kb[:, 0:128], fr(TK[:, c0:c0 + 128]),
                                     fr(diagKI[:]), start=True, stop=True,
                                     skip_group_check=True)
                    nc.tensor.matmul(kb[:, 128:256], dzT[:, c0:c0 + 128],
                                     ident[:], is_transpose=True, start=True,
                                     stop=True, skip_group_check=True)
                    nc.tensor.matmul(kb[:, 256:384], HT[:, c0:c0 + 128],
                                     ident[:], is_transpose=True, start=True,
                                     stop=True, skip_group_check=True)

                def consume(j):
                    kb = kbufs[j % 2]
                    nc.scalar.activation(keysno[:, 65 * j:65 * j + 64],
                                         kb[:, 0:64], ACTF.Copy, bias=0.0)
                    nc.scalar.activation(hno[:, 129 * j:129 * j + 128],
                                         kb[:, 256:384], ACTF.Copy, bias=0.0)
                    nc.scalar.activation(lsq[:, 64 * j:64 * j + 64],
                                         kb[:, 64:128], ACTF.Square,
                                         bias=zb[:],
                                         accum_out=losspart[:, j:j + 1])

                    v.tensor_scalar(rbdw[:, 128 * j:128 * j + 64],
                                    kb[:, 64:128],
                                    blob[:, C_BC + j:C_BC + j + 1], None,
                                    ALU.mult)
                    v.tensor_scalar(rbdw[:, 128 * j + 64:128 * j + 128],
                                    kb[:, 64:128],
                                    blob[:, C_DC + j:C_DC + j + 1], None,
                                    ALU.mult)
                    v.tensor_scalar(bdz[:, 256 * j:256 * j + 128],
                                    kb[:, 128:256],
                                    blob[:, C_BC + j:C_BC + j + 1], None,
                                    ALU.mult)
                    v.tensor_scalar(bdz[:, 256 * j + 128:256 * j + 256],
                                    kb[:, 128:256],
                                    blob[:, C_DC + j:C_DC + j + 1], None,
                                    ALU.mult)

                def gws(j):
                    nc.tensor.matmul(gw2[:],
                                     fr(rbdw[:, 128 * j:128 * j + 128]),
                                     fr(hno[:, 129 * j:129 * j + 129]),
                                     start=(j == 0), stop=(j == 3),
                                     skip_group_check=True)
                    nc.tensor.matmul(gw1[:],
                                     fr(keysno[:, 65 * j:65 * j + 65]),
                                     fr(bdz[:, 256 * j:256 * j + 256]),
                                     start=(j == 0), stop=(j == 3),
                                     skip_group_check=True)

                transposes(0)
                consume(0)
                transposes(1)
                gws(0)
                consume(1)
                transposes(2)
                gws(1)
                consume(2)
                transposes(3)
                gws(2)
                consume(3)
                gws(3)

            if stage >= 7:
                v.scalar_tensor_tensor(t1[:], blob[0:64, C_MW2:C_MW2 + 129],
                                       p_T, gw2[0:64, :], ALU.mult, ALU.add)
                v.scalar_tensor_tensor(w2pack[0:64, :],
                                       blob[0:64, C_SW2A:C_SW2A + 129], A_T,
                                       t1[:], ALU.mult, ALU.add)
                v.scalar_tensor_tensor(w2pack[64:128, :],
                                       blob[64:128, C_SW2B:C_SW2B + 129], q_T,
                                       gw2[64:128, :], ALU.mult, ALU.add)
                v.scalar_tensor_tensor(tw1[:], blob[0:65, C_W1M:C_W1M + 128],
                                       p_T, gw1[:, 0:128], ALU.mult, ALU.add)
                v.scalar_tensor_tensor(w1pack[:, 0:128],
                                       blob[0:65, C_W1S:C_W1S + 128], A_T,
                                       tw1[:], ALU.mult, ALU.add)
                v.scalar_tensor_tensor(w1pack[:, 128:256],
                                       blob[0:65, C_W1S:C_W1S + 128], q_T,
                                       gw1[:, 128:256], ALU.mult, ALU.add)
            else:
                nc.gpsimd.memset(w2pack[:], 0.0)
                nc.gpsimd.memset(w1pack[:], 0.0)
                nc.gpsimd.memset(losspart[:], 0.0)

            nc.sync.dma_start(w2pack_d[:], w2pack[:])
            nc.sync.dma_start(w1pack_d[:], w1pack[:])
            nc.sync.dma_start(loss_d[:], losspart[:])

    nc.compile()
    return nc


_NC = None


def _get_nc():
    global _NC
    if _NC is None:
        _NC = _build_nc()
    return _NC


def _pack_blobs(x, WK, WV, W1, b1, W2, b2, SW1, Sb1, SW2, Sb2):
    Bc, Dc, _, _, _ = _coeffs()
    WKV = np.concatenate([WK.T, WV.T], axis=1).astype(np.float32)
    Bct = np.ascontiguousarray(Bc.reshape(4, 128).T)
    Dct = np.ascontiguousarray(Dc.reshape(4, 128).T)
    blobs = []
    for b in range(B):
        bl = np.zeros((128, NCOL), np.float32)
        xT = x[b].T
        bl[0:64, 0:256] = xT[:, 0:256]
        bl[64:128, 0:256] = xT[:, 256:512]
        bl[0:64, C_WKV:C_W2T] = WKV
        bl[64:128, C_WKV:C_W2T] = WKV
        bl[:, C_W2T:C_B1] = W2[b].T
        bl[:, C_B1] = b1[b]
        bl[:, C_BC:C_BC + 4] = Bct
        bl[:, C_DC:C_DC + 4] = Dct
        bl[0:64, C_W1T:C_W1T + 128] = W1[b].T
        bl[0:64, C_MW2:C_MW2 + 128] = W2[b]
        bl[0:64, C_MW2 + 128] = b2[b]
        bl[0:64, C_SW2A:C_SW2A + 128] = SW2[b]
        bl[0:64, C_SW2A + 128] = Sb2[b]
        bl[64:128, C_W2N:C_W2N + 128] = W2[b]
        bl[64:128, C_B2] = b2[b]
        bl[64:128, C_SW2B:C_SW2B + 128] = SW2[b]
        bl[64:128, C_SW2B + 128] = Sb2[b]
        bl[0:64, C_W1M:C_W1M + 128] = W1[b].T
        bl[64, C_W1M:C_W1M + 128] = b1[b]
        bl[0:64, C_W1S:C_W1S + 128] = SW1[b].T
        bl[64, C_W1S:C_W1S + 128] = Sb1[b]
        blobs.append(bl)
    return blobs


def _unpack(results):
    losses = np.zeros((128, 4), np.float32)
    M_W1 = np.empty((B, H, E), np.float32)
    M_b1 = np.empty((B, H), np.float32)
    M_W2 = np.empty((B, E, H), np.float32)
    M_b2 = np.empty((B, E), np.float32)
    S_W1 = np.empty((B, H, E), np.float32)
    S_b1 = np.empty((B, H), np.float32)
    S_W2 = np.empty((B, E, H), np.float32)
    S_b2 = np.empty((B, E), np.float32)
    for b in range(B):
        r = results[b]
        w2p = np.asarray(r["w2pack"])
        w1p = np.asarray(r["w1pack"])
        losses += np.asarray(r["loss"])
        M_W2[b] = w2p[0:64, 0:128]
        M_b2[b] = w2p[0:64, 128]
        S_W2[b] = w2p[64:128, 0:128]
        S_b2[b] = w2p[64:128, 128]
        M_W1[b] = w1p[0:64, 0:128].T
        M_b1[b] = w1p[64, 0:128]
        S_W1[b] = w1p[0:64, 128:256].T
        S_b1[b] = w1p[64, 128:256]
    losses_flat = (losses.T.ravel() * np.float32(1.0 / 512.0)).astype(np.float32)
    return np.concatenate([
        losses_flat,
        M_W1.ravel(), M_b1.ravel(), M_W2.ravel(), M_b2.ravel(),
        S_W1.ravel(), S_b1.ravel(), S_W2.ravel(), S_b2.ravel(),
    ]).astype(np.float32)


def kernel(**inputs):
    nc = _get_nc()
    blobs = _pack_blobs(**{k: np.asarray(inputs[k], np.float32) for k in
                           ("x", "WK", "WV", "W1", "b1", "W2", "b2",
                            "SW1", "Sb1", "SW2", "Sb2")})
    in_maps = [{"blob": bl} for bl in blobs]
    res = run_bass_kernel_spmd(nc, in_maps, list(range(B)))
    return _unpack(res.results)


# revision 35
# speedup vs baseline: 11246.6204x; 1.0088x over previous
import sys

import numpy as np

if "/opt/trn_rl_repo" not in sys.path:
    sys.path.insert(0, "/opt/trn_rl_repo")

import concourse.bass as bass
import concourse.tile as tile
from concourse import bacc, masks, mybir
from concourse.bass_utils import run_bass_kernel_spmd

B, T, E, H = 8, 512, 64, 128
FP = mybir.dt.float32
I32 = mybir.dt.int32
ALU = mybir.AluOpType
ACTF = mybir.ActivationFunctionType

NCOL = 1099
C_XT = 0       # (128, 256) x[b].T folded: rows 0:64 toks 0:256, rows 64:128 toks 256:512
C_WKV = 256    # (128, 128) [WK.T | WV.T], duplicated in both row halves
C_W2T = 384    # (128, 64)  W2[b].T
C_B1 = 448     # (128, 1)   b1[b]
C_BC = 449     # (128, 4)   Bc'[128j+p] at [p, j]
C_DC = 453     # (128, 4)   Dc'
C_W1T = 457    # rows 0:64  (64,128) W1[b].T
C_MW2 = 585    # rows 0:64  (64,129) [W2[b] | b2[b]]
C_SW2A = 714   # rows 0:64  (64,129) [SW2[b] | Sb2[b]]
C_W2N = 457    # rows 64:128 (64,128) W2[b]
C_B2 = 585     # rows 64:128 (64,1)  b2[b]
C_SW2B = 586   # rows 64:128 (64,129) [SW2[b] | Sb2[b]]
C_W1M = 843    # rows 0:65  (65,128) [W1[b].T ; b1[b]]
C_W1S = 971    # rows 0:65  (65,128) [SW1[b].T ; Sb1[b]]

MAGIC = float(0x5F3759DF)


def _coeffs():
    beta = np.float32(1.0 - 0.999)
    eta = np.float32(0.95)
    theta = np.float32(0.05)
    beta_vec = np.full(T, beta, np.float32)
    eta_vec = np.full(T, eta, np.float32)
    p_prefix = np.cumprod(beta_vec, dtype=np.float32)
    p_suffix = np.cumprod(beta_vec[::-1], dtype=np.float32)[::-1]
    q_prefix = np.cumprod(eta_vec, dtype=np.float32)
    q_suffix = np.cumprod(eta_vec[::-1], dtype=np.float32)[::-1]
    p_T = np.float32(p_prefix[-1])
    q_T = np.float32(q_prefix[-1])
    w = (p_suffix / beta_vec * q_prefix).astype(np.float32)
    A_T = np.float32(w.sum(dtype=np.float32))
    partial = np.cumsum(w[::-1], dtype=np.float32)[::-1].astype(np.float32)
    Bco = (-theta * partial / q_prefix).astype(np.float32)
    Dco = (-theta * q_suffix).astype(np.float32)
    s = np.float32(2.0 / 512.0)  # grad-of-mean factor, exact power of two
    return (Bco * s).astype(np.float32), (Dco * s).astype(np.float32), \
        float(p_T), float(A_T), float(q_T)


# float32r matmuls fault on this runtime (verified via minimal repro);
# keep fp32 — fr is a hook left as identity.
def fr(ap):
    return ap


def _build_nc(stage=7):
    _, _, p_T, A_T, q_T = _coeffs()
    nc = bacc.Bacc(None, target_bir_lowering=False, debug=False)

    blob_d = nc.declare_dram_parameter("blob", [128, NCOL], FP, isOutput=False)
    w2pack_d = nc.declare_dram_parameter("w2pack", [128, 129], FP, isOutput=True)
    w1pack_d = nc.declare_dram_parameter("w1pack", [65, 256], FP, isOutput=True)
    loss_d = nc.declare_dram_parameter("loss", [128, 4], FP, isOutput=True)

    with tile.TileContext(nc) as tc:
        with (
            tc.tile_pool(name="sb", bufs=1) as sb,
            tc.tile_pool(name="ps", bufs=1, space=bass.MemorySpace.PSUM) as ps,
        ):
            blob = sb.tile([128, NCOL], FP)
            TK = sb.tile([128, T], FP)
            vT = sb.tile([128, T], FP)
            HT = sb.tile([128, T], FP)
            dsl = sb.tile([128, T], FP)
            dzT = sb.tile([128, T], FP)
            sqs = sb.tile([128, T], FP)
            ssq = sb.tile([128, 1], FP)
            tnr = sb.tile([128, 1], FP)
            ya = sb.tile([128, 1], FP)
            yb = sb.tile([128, 1], FP)
            t2 = sb.tile([128, 1], FP)
            f2 = sb.tile([128, 1], FP)
            halfx = sb.tile([128, 1], FP)
            c15 = sb.tile([128, 1], FP)
            zb = sb.tile([128, 1], FP)
            ident = sb.tile([128, 128], FP)
            diagKI = sb.tile([128, 128], FP)
            W1Tn = sb.tile([64, 128], FP)
            keysno = sb.tile([128, 4 * 65], FP)
            hno = sb.tile([128, 4 * 129], FP)
            rbdw = sb.tile([128, 512], FP)
            bdz = sb.tile([128, 1024], FP)
            lsq = sb.tile([128, 256], FP)
            losspart = sb.tile([128, 4], FP)
            w2pack = sb.tile([128, 129], FP)
            w1pack = sb.tile([65, 256], FP)
            t1 = sb.tile([64, 129], FP)
            tw1 = sb.tile([65, 128], FP)

            gw2 = ps.tile([128, 129], FP, tag="gw2")
            gw1 = ps.tile([65, 256], FP, tag="gw1")

            nc.sync.dma_start(blob[:, 0:C_W2T], blob_d[:, 0:C_W2T])
            nc.sync.dma_start(blob[:, C_W2T:NCOL], blob_d[:, C_W2T:NCOL])

            masks.make_identity(nc, ident[:])
            nc.vector.memset(zb[:], 0.0)
            nc.vector.memset(keysno[:], 1.0)
            nc.vector.memset(hno[:], 1.0)

            v = nc.vector
            g = nc.gpsimd
            # rn-independent, so do it up front, off the post-Newton path
            v.tensor_copy(diagKI[64:128, :], ident[64:128, :])

            if stage >= 2:
                # HW constraint (found empirically): a PSUM bank must only be
                # written by matmuls of a single tile_position, so the two
                # halves (positions (0,0) and (64,0)) go to separate banks.
                zkva = ps.tile([128, 256], FP, tag="zkva", bufs=1)
                zkvb = ps.tile([128, 256], FP, tag="zkvb", bufs=1)
                nc.tensor.matmul(zkva[:], fr(blob[0:64, C_WKV:C_W2T]),
                                 fr(blob[0:64, C_XT:C_XT + 256]),
                                 start=True, stop=True)
                nc.tensor.matmul(zkvb[:], fr(blob[64:128, C_WKV:C_W2T]),
                                 fr(blob[64:128, C_XT:C_XT + 256]),
                                 start=True, stop=True)

                # ACT order keeps exactly two table loads: all Silu first, one
                # Derivative_silu, then Copy/Square (present in every table).
                nc.scalar.activation(TK[:, 0:256], zkva[:], ACTF.Silu,
                                     bias=zb[:])
                nc.scalar.activation(TK[:, 256:512], zkvb[:], ACTF.Silu,
                                     bias=zb[:])
                nc.scalar.activation(sqs[:], TK[:], ACTF.Square, bias=zb[:],
                                     accum_out=ssq[:])

            if stage >= 3:
                # rsqrt(ssq) via bit-trick seed + 3 Newton iterations (DVE
                # only; Sqrt ACT funcs live in a third table -> extra load)
                v.tensor_scalar(tnr[:].bitcast(I32), ssq[:].bitcast(I32), 1,
                                None, ALU.logical_shift_right)
                v.tensor_scalar(ya[:].bitcast(I32), tnr[:].bitcast(I32), MAGIC,
                                -1.0, ALU.subtract, ALU.mult)
                v.tensor_scalar(halfx[:], ssq[:], -0.5, None, ALU.mult)
                y = ya
                for _ in range(3):
                    yn = yb if y is ya else ya
                    # y' = (y*halfx*y + 1.5)*y, two fused tensor_scalar ops
                    v.tensor_scalar(t2[:], y[:], halfx[:, :], y[:, :],
                                    ALU.mult, ALU.mult)
                    v.tensor_scalar(yn[:], t2[:], 1.5, y[:, :],
                                    ALU.add, ALU.mult)
                    y = yn
                rn = y

                # W1Tn first: it gates the z1 matmul; the other scales can
                # run while the PE is busy.
                v.tensor_scalar(W1Tn[:], blob[0:64, C_W1T:C_W1T + 128],
                                rn[0:64, :], None, ALU.mult)
                v.tensor_scalar(diagKI[0:64, :], ident[0:64, :], rn[0:64, :],
                                None, ALU.mult)
                v.tensor_scalar(vT[64:128, :], TK[64:128, :], rn[64:128, :],
                                None, ALU.mult)

            if stage >= 4:
                # Forward/backward chain split into token halves. PSUM dep
                # tracking is tile-granular, so each half gets its own TILE
                # (distinct bank): half-a consumers start without waiting for
                # half-b producers. Dead K/V banks are recycled (positions
                # match: zkva (0,0) hosts z1b, zkvb (64,0) hosts dha).
                z1 = ps.tile([128, T], FP, tag="z1", bufs=1)
                kdh = ps.tile([128, 384], FP, tag="kdh", bufs=1)
                z1h = [z1[:, 0:256], zkva[:]]
                # yps halves live in rows 0:64 of the gw1/kdh banks (both
                # (0,0)-position, dead until after the residual reads them)
                yph = [gw1[0:64, 0:256], kdh[0:64, 0:256]]
                dhb = ps.tile([128, 256], FP, tag="dh", bufs=1)
                dhh = [zkvb[:], dhb[:]]
                HVS = [(0, 256), (256, 512)]
                for i, (a, b) in enumerate(HVS):
                    nc.tensor.matmul(z1h[i][:, 0:256], fr(W1Tn[:]),
                                     fr(TK[0:64, a:b]), start=True, stop=True,
                                     skip_group_check=True)
                if stage >= 5:
                    for i, (a, b) in enumerate(HVS):
                        nc.scalar.activation(HT[:, a:b], z1h[i][:, 0:256],
                                             ACTF.Silu,
                                             bias=blob[:, C_B1:C_B1 + 1])
                        nc.tensor.matmul(yph[i][:, :],
                                         fr(blob[:, C_W2T:C_W2T + 64]),
                                         fr(HT[:, a:b]), start=True, stop=True,
                                         skip_group_check=True)
                    # Both Derivative_silu ops stay contiguous: exactly one
                    # table switch away from Silu, and dzT is consumed late.
                    for i, (a, b) in enumerate(HVS):
                        nc.scalar.activation(dsl[:, a:b], z1h[i][:, 0:256],
                                             ACTF.Derivative_silu,
                                             bias=blob[:, C_B1:C_B1 + 1])
                    for i, (a, b) in enumerate(HVS):
                        # residual R = (y + b2) - v_norm, overwrites V rows;
                        # in0 reads partitions 0:64 while out/in1 sit at
                        # 64:128 (DVE operands carry independent bases)
                        v.scalar_tensor_tensor(TK[64:128, a:b],
                                               yph[i][:, :],
                                               blob[64:128, C_B2:C_B2 + 1],
                                               vT[64:128, a:b], ALU.add,
                                               ALU.subtract)
                        nc.tensor.matmul(dhh[i][:, 0:256],
                                         fr(blob[64:128, C_W2N:C_W2N + 128]),
                                         fr(TK[64:128, a:b]), start=True,
                                         stop=True, skip_group_check=True)
                    for i, (a, b) in enumerate(HVS):
                        v.tensor_tensor(dzT[:, a:b], dhh[i][:, 0:256],
                                        dsl[:, a:b], ALU.mult)
                else:
                    nc.scalar.activation(HT[:, 0:256], z1h[0][:, 0:256],
                                         ACTF.Silu,
                                         bias=blob[:, C_B1:C_B1 + 1])
                    nc.scalar.activation(HT[:, 256:512], z1h[1][:, 0:256],
                                         ACTF.Silu,
                                         bias=blob[:, C_B1:C_B1 + 1])
                    nc.scalar.activation(dsl[:, 0:256], z1h[0][:, 0:256],
                                         ACTF.Derivative_silu,
                                         bias=blob[:, C_B1:C_B1 + 1])
                    nc.scalar.activation(dsl[:, 256:512], z1h[1][:, 0:256],
                                         ACTF.Derivative_silu,
                                         bias=blob[:, C_B1:C_B1 + 1])

            if stage >= 6:
                # Software pipeline over the 4 token chunks. kdh double-
                # buffers between its own bank and a slice of the (dead) z1
                # bank — both banks are written only by tile-position (0,0)
                # matmuls, keeping the single-position-per-bank rule.
                kbufs = [kdh, z1[:, 0:384]]

                def transposes(j):
                    kb = kbufs[j % 2]
                    c0 = j * 128
                    nc.tensor.matmul(kb[:, 0:128], fr(TK[:, c0:c0 + 128]),
                                     fr(diagKI[:]), start=True, stop=True,
                                     skip_group_check=True)
                    nc.tensor.matmul(kb[:, 128:256], dzT[:, c0:c0 + 128],
                                     ident[:], is_transpose=True, start=True,
                                     stop=True, skip_group_check=True)
                    nc.tensor.matmul(kb[:, 256:384], HT[:, c0:c0 + 128],
                                     ident[:], is_transpose=True, start=True,
                                     stop=True, skip_group_check=True)

                def consume(j):
                    kb = kbufs[j % 2]
                    nc.scalar.activation(keysno[:, 65 * j:65 * j + 64],
                                         kb[:, 0:64], ACTF.Copy, bias=0.0)
                    nc.scalar.activation(hno[:, 129 * j:129 * j + 128],
                                         kb[:, 256:384], ACTF.Copy, bias=0.0)
                    nc.scalar.activation(lsq[:, 64 * j:64 * j + 64],
                                         kb[:, 64:128], ACTF.Square,
                                         bias=zb[:],
                                         accum_out=losspart[:, j:j + 1])

                    v.tensor_scalar(rbdw[:, 128 * j:128 * j + 64],
                                    kb[:, 64:128],
                                    blob[:, C_BC + j:C_BC + j + 1], None,
                                    ALU.mult)
                    v.tensor_scalar(rbdw[:, 128 * j + 64:128 * j + 128],
                                    kb[:, 64:128],
                                    blob[:, C_DC + j:C_DC + j + 1], None,
                                    ALU.mult)
                    v.tensor_scalar(bdz[:, 256 * j:256 * j + 128],
                                    kb[:, 128:256],
                                    blob[:, C_BC + j:C_BC + j + 1], None,
                                    ALU.mult)
                    v.tensor_scalar(bdz[:, 256 * j + 128:256 * j + 256],
                                    kb[:, 128:256],
                                    blob[:, C_DC + j:C_DC + j + 1], None,
                                    ALU.mult)

                def gws(j):
                    nc.tensor.matmul(gw2[:],
                                     fr(rbdw[:, 128 * j:128 * j + 128]),
                                     fr(hno[:, 129 * j:129 * j + 129]),
                                     start=(j == 0), stop=(j == 3),
                                     skip_group_check=True)
                    nc.tensor.matmul(gw1[:],
                                     fr(keysno[:, 65 * j:65 * j + 65]),
                                     fr(bdz[:, 256 * j:256 * j + 256]),
                                     start=(j == 0), stop=(j == 3),
                                     skip_group_check=True)

                transposes(0)
                consume(0)
                transposes(1)
                gws(0)
                consume(1)
                transposes(2)
                gws(1)
                consume(2)
                transposes(3)
                gws(2)
                consume(3)
                gws(3)

            if stage >= 7:
                v.scalar_tensor_tensor(t1[:], blob[0:64, C_MW2:C_MW2 + 129],
                                       p_T, gw2[0:64, :], ALU.mult, ALU.add)
                v.scalar_tensor_tensor(w2pack[0:64, :],
                                       blob[0:64, C_SW2A:C_SW2A + 129], A_T,
                                       t1[:], ALU.mult, ALU.add)
                v.scalar_tensor_tensor(w2pack[64:128, :],
                                       blob[64:128, C_SW2B:C_SW2B + 129], q_T,
                                       gw2[64:128, :], ALU.mult, ALU.add)
                v.scalar_tensor_tensor(tw1[:], blob[0:65, C_W1M:C_W1M + 128],
                                       p_T, gw1[:, 0:128], ALU.mult, ALU.add)
                v.scalar_tensor_tensor(w1pack[:, 0:128],
                                       blob[0:65, C_W1S:C_W1S + 128], A_T,
                                       tw1[:], ALU.mult, ALU.add)
                v.scalar_tensor_tensor(w1pack[:, 128:256],
                                       blob[0:65, C_W1S:C_W1S + 128], q_T,
                                       gw1[:, 128:256], ALU.mult, ALU.add)
            else:
                nc.gpsimd.memset(w2pack[:], 0.0)
                nc.gpsimd.memset(w1pack[:], 0.0)
                nc.gpsimd.memset(losspart[:], 0.0)

            nc.sync.dma_start(w2pack_d[:], w2pack[:])
            nc.sync.dma_start(w1pack_d[:], w1pack[:])
            nc.sync.dma_start(loss_d[:], losspart[:])

    nc.compile()
    return nc


_NC = None


def _get_nc():
    global _NC
    if _NC is None:
        _NC = _build_nc()
    return _NC


def _pack_blobs(x, WK, WV, W1, b1, W2, b2, SW1, Sb1, SW2, Sb2):
    Bc, Dc, _, _, _ = _coeffs()
    WKV = np.concatenate([WK.T, WV.T], axis=1).astype(np.float32)
    Bct = np.ascontiguousarray(Bc.reshape(4, 128).T)
    Dct = np.ascontiguousarray(Dc.reshape(4, 128).T)
    blobs = []
    for b in range(B):
        bl = np.zeros((128, NCOL), np.float32)
        xT = x[b].T
        bl[0:64, 0:256] = xT[:, 0:256]
        bl[64:128, 0:256] = xT[:, 256:512]
        bl[0:64, C_WKV:C_W2T] = WKV
        bl[64:128, C_WKV:C_W2T] = WKV
        bl[:, C_W2T:C_B1] = W2[b].T
        bl[:, C_B1] = b1[b]
        bl[:, C_BC:C_BC + 4] = Bct
        bl[:, C_DC:C_DC + 4] = Dct
        bl[0:64, C_W1T:C_W1T + 128] = W1[b].T
        bl[0:64, C_MW2:C_MW2 + 128] = W2[b]
        bl[0:64, C_MW2 + 128] = b2[b]
        bl[0:64, C_SW2A:C_SW2A + 128] = SW2[b]
        bl[0:64, C_SW2A + 128] = Sb2[b]
        bl[64:128, C_W2N:C_W2N + 128] = W2[b]
        bl[64:128, C_B2] = b2[b]
        bl[64:128, C_SW2B:C_SW2B + 128] = SW2[b]
        bl[64:128, C_SW2B + 128] = Sb2[b]
        bl[0:64, C_W1M:C_W1M + 128] = W1[b].T
        bl[64, C_W1M:C_W1M + 128] = b1[b]
        bl[0:64, C_W1S:C_W1S + 128] = SW1[b].T
        bl[64, C_W1S:C_W1S + 128] = Sb1[b]
        blobs.append(bl)
    return blobs


def _unpack(results):
    losses = np.zeros((128, 4), np.float32)
    M_W1 = np.empty((B, H, E), np.float32)
    M_b1 = np.empty((B, H), np.float32)
    M_W2 = np.empty((B, E, H), np.float32)
    M_b2 = np.empty((B, E), np.float32)
    S_W1 = np.empty((B, H, E), np.float32)
    S_b1 = np.empty((B, H), np.float32)
    S_W2 = np.empty((B, E, H), np.float32)
    S_b2 = np.empty((B, E), np.float32)
    for b in range(B):
        r = results[b]
        w2p = np.asarray(r["w2pack"])
        w1p = np.asarray(r["w1pack"])
        losses += np.asarray(r["loss"])
        M_W2[b] = w2p[0:64, 0:128]
        M_b2[b] = w2p[0:64, 128]
        S_W2[b] = w2p[64:128, 0:128]
        S_b2[b] = w2p[64:128, 128]
        M_W1[b] = w1p[0:64, 0:128].T
        M_b1[b] = w1p[64, 0:128]
        S_W1[b] = w1p[0:64, 128:256].T
        S_b1[b] = w1p[64, 128:256]
    losses_flat = (losses.T.ravel() * np.float32(1.0 / 512.0)).astype(np.float32)
    return np.concatenate([
        losses_flat,
        M_W1.ravel(), M_b1.ravel(), M_W2.ravel(), M_b2.ravel(),
        S_W1.ravel(), S_b1.ravel(), S_W2.ravel(), S_b2.ravel(),
    ]).astype(np.float32)


def kernel(**inputs):
    nc = _get_nc()
    blobs = _pack_blobs(**{k: np.asarray(inputs[k], np.float32) for k in
                           ("x", "WK", "WV", "W1", "b1", "W2", "b2",
                            "SW1", "Sb1", "SW2", "Sb2")})
    in_maps = [{"blob": bl} for bl in blobs]
    res = run_bass_kernel_spmd(nc, in_maps, list(range(B)))
    return _unpack(res.results)
